# revision 32
# baseline (speedup 1.0000x reference)
"""Trainium2 Bass kernel for nn_AttentiveGatingv2 (moe_routing).

Reference computation (shapes hardcoded):
  x: [64, 96, 207, 64] -> take last 6 timesteps -> per-(b,n) token:
  z = proj(x_k); qkv = in_proj(z); 4-head attention over the 6 steps;
  out-proj; mean over steps; fc to 8 experts; softmax -> [64, 207, 8].

Host-side algebraic fusion (validated vs reference in fp-faithful sim):
  W_eff = in_proj_w @ proj_w  (96x64), b_eff folded via an appended ones
  row (q-rows pre-scaled by 1/sqrt(8)).  Since mean-over-steps commutes
  with the linear out-proj/fc, and  ctxbar^{hc} = sum_j wbar_j^h v_j^{hc},
  the whole post-attention stack collapses INTO the qkv matmul weights:
    logits_e = sum_{j,h} wbar_j^h * ghv_j^{h,e} ,
    ghv columns = (G_h W_v_h) x_aug  with  G = fc_w@out_w/6.
  So the single PE matmul per (tile, step) produces q(32) | k(32) | ghv(32)
  and NO post-attention matmul/transpose is needed; the expert logits are
  one bf16 2x-mode DVE mul + an add-tree/reduce.  exp(g_b) is folded in as
  a broadcast multiply before the final softmax.

Engine strategy (measured progression on trn2: v1 58.1us -> ghv-fold
restructure 46.6 -> wa-pack/tail polish 46.2 -> 2-queue loads 45.9us;
gpsimd compute offloads measured ~520ns/op dispatch overhead and
regressed to 49-49.6us twice, so ALL elementwise stays on VectorE):
  - scores/softmax on VectorE in bf16, hot ops in 2x mode (step-1
    innermost APs; broadcasts only on middle dims); strided reduce_sum
    replaced by bf16 add-trees; reciprocal_approx_fast (~5x).
  - ghv evacuated PSUM->SBUF transposed to [e, j, h] so the logits mul
    runs 2x with a contiguous reduce.
  - x streamed in 7 chunks ALTERNATING between the Sync and GpSimd DMA
    queues (two engines transfer in parallel; a single-queue stream
    gated tile-1 matmuls by 2us); first two chunks are single tiles,
    wa packed into chunk 0 so one DMA gates the first matmul.
  - software pipelining ACROSS tile groups {4,4,5}: VectorE emission
    A0 A1 [tmp(t8)] C0 A2' C1 E0 C2 E1 E2 (A=scores, C=softmax+logits,
    E=final softmax) so the strict per-engine FIFO never stalls on the
    ScalarE exp handoffs; tile-8's score-mul is emitted between A1 and
    C0 as filler for the measured 1.2us gap where C0 waits on the
    group-0 exp (itself queued behind group-2 evacuations); output
    stores on the Sync HWDGE queue.
Final: HW exec 45043 ns (max over cores), rel err 8.26e-4.
8 NeuronCores data-parallel over batch; no cross-device communication.
"""

import numpy as np
import ml_dtypes

import concourse.bass as bass
import concourse.mybir as mybir
import concourse.tile as tile
from concourse.bacc import Bacc
from concourse.bass_utils import run_bass_kernel_spmd

F32 = mybir.dt.float32
BF16 = mybir.dt.bfloat16
NP_BF16 = ml_dtypes.bfloat16

# problem dims
B, T, NTOK, C = 64, 96, 207, 64
D, H, HD, K = 32, 4, 8, 6
E = 8
NCORES = 8

# per-core dims
B_SH = B // NCORES            # 8
S = B_SH * NTOK               # 1656 tokens per core
P = 128
NT = (S + P - 1) // P         # 13 tiles
S_PAD = NT * P                # 1664
CA = C + 1                    # 65: channels + ones row
E3 = 3 * D                    # 96 matmul output cols: q(32)|k(32)|ghv(32)
KK = K * K                    # 36
GQ = 5                        # max tiles per group (groups are 4,4,5)
TKP = K * P                   # elems per tile in the packed x layout

GROUPS = [(0, 4), (4, 4), (8, 5)]
# (start_tile, n_tiles, queue): alternate sync/gpsimd so two DMA engines
# stream x in parallel; chunk 0 also carries wa.
LOADS = [(0, 1, "sync"), (1, 1, "gpsimd"), (2, 2, "sync"), (4, 2, "gpsimd"),
         (6, 2, "sync"), (8, 2, "gpsimd"), (10, 3, "sync")]


def _build_module():
    nc = Bacc()

    # xt packs wa (96 cols) followed by NT tiles of [K, P] bf16 features
    xt = nc.dram_tensor("xt", [CA, E3 + NT * TKP], BF16, kind="ExternalInput")
    cf = nc.dram_tensor("cf", [P, E], F32, kind="ExternalInput")  # exp(g_b)
    # out[p, t, e]: token (t*128+p); host reassembles.
    out = nc.dram_tensor("out", [P, NT, E], F32, kind="ExternalOutput")

    AF = mybir.ActivationFunctionType
    AX = mybir.AxisListType

    def apv(t, dims, extra_offset=0):
        # custom AP over tile t: keep t's partition dim, replace free dims
        return bass.AP(
            tensor=t.tensor,
            offset=t.offset + extra_offset,
            ap=[list(t.ap[0])] + [list(d) for d in dims],
        )

    with tile.TileContext(nc) as tc:
        with (
            tc.tile_pool(name="singles", bufs=1) as singles,
            tc.tile_pool(name="xload", bufs=1) as xload,
            tc.tile_pool(name="work", bufs=3) as work,
            tc.tile_pool(name="psum", bufs=3, space="PSUM") as psum,
        ):
            cf_sb = singles.tile([P, E], F32)
            nc.scalar.dma_start(out=cf_sb, in_=cf[:, :])
            dum = singles.tile([P, 1], F32)
            nc.scalar.activation(out=dum, in_=cf_sb[:, 0:1], func=AF.Exp)

            xg_tiles = []
            xg_dmas = []
            for li, (ts, nl, qname) in enumerate(LOADS):
                q = nc.sync if qname == "sync" else nc.gpsimd
                if li == 0:
                    xg_sb = xload.tile([CA, E3 + TKP], BF16, name="xg0")
                    xd = q.dma_start(out=xg_sb, in_=xt[:, 0:E3 + TKP])
                else:
                    xg_sb = xload.tile([CA, nl, K, P], BF16, name=f"xg{li}")
                    xd = q.dma_start(
                        out=xg_sb,
                        in_=xt[:, E3 + ts * TKP:E3 + (ts + nl) * TKP])
                xg_tiles.append(xg_sb)
                xg_dmas.append(xd)
            wa_sb = xg_tiles[0][:, 0:E3]

            out_sb = singles.tile([P, NT, E], F32)

            # ---- per-group tiles (bufs=3 -> one set per group) ----
            def group_tiles(gi):
                t = {}
                t["qk"] = work.tile([P, GQ, K, 2 * D], BF16, name="qk")
                t["gb"] = work.tile([P, GQ, E, K, H], BF16, name="gb")
                t["tmp"] = work.tile([P, GQ, KK, D], BF16, name="tmp")
                t["s1"] = work.tile([P, GQ, KK, H, 4], BF16, name="s1")
                t["s2"] = work.tile([P, GQ, KK, H, 2], BF16, name="s2")
                t["sc"] = work.tile([P, GQ, KK, H], F32, name="sc")
                t["es"] = work.tile([P, GQ, K, K, H], BF16, name="es")
                t["zt1"] = work.tile([P, GQ, K, 3, H], BF16, name="zt1")
                t["zs2"] = work.tile([P, GQ, K, H], BF16, name="zs2")
                t["zs"] = work.tile([P, GQ, K, H], F32, name="zs")
                t["rs32"] = work.tile([P, GQ, K, H], F32, name="rs32")
                t["rs16"] = work.tile([P, GQ, K, H], BF16, name="rs16")
                t["at"] = work.tile([P, GQ, K, K, H], BF16, name="at")
                t["wb1"] = work.tile([P, GQ, 3, K, H], BF16, name="wb1")
                t["wb2"] = work.tile([P, GQ, K, H], BF16, name="wb2")
                t["wbar"] = work.tile([P, GQ, K, H], BF16, name="wbar")
                t["lg1"] = work.tile([P, GQ, E, K, H], BF16, name="lg1")
                t["lgt"] = work.tile([P, GQ, E, 12], BF16, name="lgt")
                t["lg"] = work.tile([P, GQ, E], F32, name="lg")
                t["el"] = work.tile([P, GQ, E], F32, name="el")
                t["el2"] = work.tile([P, GQ, E], F32, name="el2")
                t["zf"] = work.tile([P, GQ], F32, name="zf")
                t["rf"] = work.tile([P, GQ], F32, name="rf")
                return t

            gts = [group_tiles(gi) for gi in range(len(GROUPS))]
            first_mm = {}           # global tile idx -> first matmul inst

            def tile_load(tix):
                for li, (ts, nl, _) in enumerate(LOADS):
                    if ts <= tix < ts + nl:
                        return li, tix - ts
                raise AssertionError

            def lhsT_of(tix, i):
                li, lidx = tile_load(tix)
                if li == 0:
                    return apv(xg_tiles[0], [[1, P]], E3 + i * P)
                return xg_tiles[li][:, lidx, i, :]

            def emit_mm_evac(gi):
                tg, g = GROUPS[gi]
                t = gts[gi]
                for u in range(g):
                    tix = tg + u
                    qkv_ps = psum.tile([P, 8, P], F32, tag="qkv_ps", bufs=3,
                                       name="qkv_ps")
                    for i in range(K):
                        mm = nc.tensor.matmul(
                            out=qkv_ps[:, i, 0:E3],
                            lhsT=lhsT_of(tix, i),
                            rhs=wa_sb,
                            start=True,
                            stop=True,
                        )
                        if i == 0 and tix not in first_mm:
                            first_mm[tix] = mm
                    # evac q,k as-is; ghv transposed to [e, j, h] so the
                    # logits mul runs in 2x mode with a contiguous reduce
                    nc.scalar.copy(out=t["qk"][:, u],
                                   in_=qkv_ps[:, 0:K, 0:2 * D])
                    nc.scalar.copy(
                        out=apv(t["gb"], [[K * H, E], [H, K], [1, H]],
                                u * E * K * H),
                        in_=apv(qkv_ps, [[1, E], [P, K], [E, H]], 2 * D))

            def emit_A_tmps(gi, us):
                t = gts[gi]
                muls = []
                for u in us:
                    off = u * K * 2 * D
                    q_ap = apv(t["qk"], [[2 * D, K], [0, K], [1, D]], off)
                    k_ap = apv(t["qk"], [[0, K], [2 * D, K], [1, D]], off + D)
                    tm_o = apv(t["tmp"], [[D, KK], [1, D]], u * KK * D)
                    muls.append(nc.vector.tensor_mul(tm_o, q_ap, k_ap))
                return muls

            def emit_A(gi, us=None):
                # scores: tmp mul per tile + bf16 add tree (s1,s2 2x; s3 1x)
                tg, g = GROUPS[gi]
                t = gts[gi]
                gKK = g * KK
                muls = emit_A_tmps(gi, range(g) if us is None else us)
                a = apv(t["tmp"], [[D, gKK], [HD, H], [1, 4]])
                b = apv(t["tmp"], [[D, gKK], [HD, H], [1, 4]], 4)
                o = apv(t["s1"], [[16, gKK], [4, H], [1, 4]])
                nc.vector.tensor_add(o, a, b)
                a = apv(t["s1"], [[16, gKK], [4, H], [1, 2]])
                b = apv(t["s1"], [[16, gKK], [4, H], [1, 2]], 2)
                o = apv(t["s2"], [[8, gKK], [2, H], [1, 2]])
                nc.vector.tensor_add(o, a, b)
                a = apv(t["s2"], [[8, gKK], [2, H]])
                b = apv(t["s2"], [[8, gKK], [2, H]], 1)
                o = apv(t["sc"], [[H, gKK], [1, H]])
                return muls[0], nc.vector.tensor_add(o, a, b)

            def emit_B(gi):
                tg, g = GROUPS[gi]
                t = gts[gi]
                nc.scalar.activation(out=t["es"][:, 0:g], in_=t["sc"][:, 0:g],
                                     func=AF.Exp)

            def emit_zs(gi, eng):
                # zs = sum_j es via bf16 tree (strided reduce is slower)
                tg, g = GROUPS[gi]
                t = gts[gi]
                gK = g * K
                a = apv(t["es"], [[24, gK], [4, 3], [1, H]])
                b = apv(t["es"], [[24, gK], [4, 3], [1, H]], 12)
                o = apv(t["zt1"], [[12, gK], [4, 3], [1, H]])
                eng.tensor_add(o, a, b)
                a = apv(t["zt1"], [[12, gK], [1, H]])
                b = apv(t["zt1"], [[12, gK], [1, H]], 4)
                o = apv(t["zs2"], [[4, gK], [1, H]])
                eng.tensor_add(o, a, b)
                a = apv(t["zs2"], [[4, gK], [1, H]])
                b = apv(t["zt1"], [[12, gK], [1, H]], 8)
                o = apv(t["zs"], [[4, gK], [1, H]])
                eng.tensor_add(o, a, b)

            def emit_Cpre(gi):
                # 1/zs, bf16 cast, attn = es * rs (all VectorE)
                tg, g = GROUPS[gi]
                t = gts[gi]
                gK = g * K
                n = gK * H
                nc.vector.reciprocal_approx_fast(
                    out=apv(t["rs32"], [[1, n]]), in_=apv(t["zs"], [[1, n]]))
                nc.vector.tensor_copy(out=apv(t["rs16"], [[1, n]]),
                                      in_=apv(t["rs32"], [[1, n]]))
                a = apv(t["es"], [[24, gK], [4, K], [1, H]])
                b = apv(t["rs16"], [[4, gK], [0, K], [1, H]])
                o = apv(t["at"], [[24, gK], [4, K], [1, H]])
                nc.vector.tensor_mul(o, a, b)

            def emit_wb(gi, eng):
                # wbar = sum_i attn via bf16 tree
                tg, g = GROUPS[gi]
                t = gts[gi]
                a = apv(t["at"], [[144, g], [24, 3], [1, K * H]])
                b = apv(t["at"], [[144, g], [24, 3], [1, K * H]], 72)
                o = apv(t["wb1"], [[72, g], [24, 3], [1, K * H]])
                eng.tensor_add(o, a, b)
                a = apv(t["wb1"], [[72, g], [1, K * H]])
                b = apv(t["wb1"], [[72, g], [1, K * H]], 24)
                o = apv(t["wb2"], [[24, g], [1, K * H]])
                eng.tensor_add(o, a, b)
                a = apv(t["wb2"], [[24, g], [1, K * H]])
                b = apv(t["wb1"], [[72, g], [1, K * H]], 48)
                o = apv(t["wbar"], [[24, g], [1, K * H]])
                eng.tensor_add(o, a, b)

            def emit_Cpost(gi):
                # logits: lg1[g, e, (j,h)] = wbar * ghv; half-tree + reduce
                tg, g = GROUPS[gi]
                t = gts[gi]
                a = apv(t["wbar"], [[24, g], [0, E], [1, K * H]])
                b = apv(t["gb"], [[K * H * E, g], [K * H, E], [1, K * H]])
                o = apv(t["lg1"], [[K * H * E, g], [K * H, E], [1, K * H]])
                nc.vector.tensor_mul(o, a, b)
                a = apv(t["lg1"], [[K * H * E, g], [K * H, E], [1, 12]])
                b = apv(t["lg1"], [[K * H * E, g], [K * H, E], [1, 12]], 12)
                o = apv(t["lgt"], [[12 * E, g], [12, E], [1, 12]])
                nc.vector.tensor_add(o, a, b)
                nc.vector.reduce_sum(
                    out=apv(t["lg"], [[E, g], [1, E]]),
                    in_=apv(t["lgt"], [[12 * E, g], [12, E], [1, 12]]),
                    axis=AX.X)

            def emit_D(gi):
                tg, g = GROUPS[gi]
                t = gts[gi]
                nc.scalar.activation(out=t["el"][:, 0:g], in_=t["lg"][:, 0:g],
                                     func=AF.Exp)

            def emit_E(gi):
                tg, g = GROUPS[gi]
                t = gts[gi]
                ebg = apv(cf_sb, [[0, g], [1, E]])
                nc.vector.tensor_mul(t["el2"][:, 0:g], t["el"][:, 0:g], ebg)
                nc.vector.reduce_sum(out=t["zf"][:, 0:g],
                                     in_=t["el2"][:, 0:g], axis=AX.X)
                nc.vector.reciprocal_approx_fast(out=t["rf"][:, 0:g],
                                                 in_=t["zf"][:, 0:g])
                rf_ap = apv(t["rf"], [[1, g], [0, E]])
                nc.vector.tensor_mul(out_sb[:, tg:tg + g, :],
                                     t["el2"][:, 0:g], rf_ap)
                nc.sync.dma_start(out=out[:, tg:tg + g, :],
                                  in_=out_sb[:, tg:tg + g, :])

            def emit_C(gi):
                # GPSIMD TT ops measured ~720ns each regardless of size
                # (dispatch+join overhead) — keep the whole chain on VectorE
                emit_zs(gi, nc.vector)
                emit_Cpre(gi)
                emit_wb(gi, nc.vector)
                emit_Cpost(gi)

            # ---- software-pipelined emission ----
            emit_mm_evac(0)
            emit_mm_evac(1)
            _, a0_s3 = emit_A(0)
            a1_tmp0, _ = emit_A(1)
            tile.add_dep_helper(a1_tmp0.ins, a0_s3.ins, sync=True,
                                reason="A0 tree before A1 tmps")
            emit_B(0)
            emit_mm_evac(2)
            emit_A_tmps(2, [0])
            emit_C(0)
            emit_A(2, us=[1, 2, 3, 4])
            emit_B(1)
            emit_C(1)
            emit_D(0)
            emit_E(0)
            emit_B(2)
            emit_C(2)
            emit_D(1)
            emit_E(1)
            emit_D(2)
            emit_E(2)

            # later loads wait on compute progress so early tiles don't
            # round-robin behind all the loads
            for li in range(3, len(LOADS)):
                gate_tile = LOADS[li - 2][0]
                tile.add_dep_helper(xg_dmas[li].ins, first_mm[gate_tile].ins,
                                    sync=True, reason="load stagger")

    nc.finalize()
    return nc


_NC = None


def _get_module():
    global _NC
    if _NC is None:
        _NC = _build_module()
    return _NC


def _host_prep(x, proj_w, proj_b, in_proj_w, in_proj_b, out_w, out_b, fc_w, fc_b):
    scale = np.float32(1.0 / np.sqrt(HD))
    w_eff = (in_proj_w @ proj_w).astype(np.float32)          # [96, 64]
    b_eff = (in_proj_w @ proj_b + in_proj_b).astype(np.float32)
    w_eff[0:D] *= scale
    b_eff[0:D] *= scale
    w_aug = np.concatenate([w_eff, b_eff[:, None]], axis=1)  # [96, 65]

    G = (fc_w @ out_w / np.float32(K)).astype(np.float32)    # [8, 32]
    g_b = (fc_w @ out_b + fc_b).astype(np.float32)

    wa = np.zeros((CA, E3), dtype=np.float32)
    wa[:, 0:2 * D] = w_aug[0:2 * D].T                        # q | k
    for h in range(H):
        wv_h = w_aug[2 * D + HD * h:2 * D + HD * (h + 1)]    # [8, 65]
        G_h = G[:, HD * h:HD * (h + 1)]                      # [8(e), 8(c)]
        wa[:, 2 * D + E * h:2 * D + E * (h + 1)] = wv_h.T @ G_h.T
    wa = wa.astype(NP_BF16)

    cf = np.broadcast_to(np.exp(g_b).astype(np.float32)[None, :],
                         (P, E)).copy()

    # x: [B, T, N, C] -> last K steps -> per-core packed [CA, 96 + NT*K*P]
    xk = x[:, T - K:, :, :]                                  # [B, K, N, C]
    in_maps = []
    for core in range(NCORES):
        xc = xk[core * B_SH:(core + 1) * B_SH]               # [8, K, N, C]
        xc = np.transpose(xc, (3, 1, 0, 2)).reshape(C, K, S)
        xp = np.zeros((C, K, S_PAD), dtype=np.float32)
        xp[:, :, 0:S] = xc
        xp = xp.reshape(C, K, NT, P).transpose(0, 2, 1, 3)   # [C, NT, K, P]
        xtc = np.empty((CA, E3 + NT * TKP), dtype=NP_BF16)
        xtc[:, 0:E3] = wa
        xfull = np.empty((CA, NT, K, P), dtype=NP_BF16)
        xfull[0:C] = xp.astype(NP_BF16)
        xfull[C] = 1
        xtc[:, E3:] = xfull.reshape(CA, NT * TKP)
        in_maps.append({"xt": xtc, "cf": cf})
    return in_maps


def kernel(x, proj_w, proj_b, in_proj_w, in_proj_b, out_w, out_b, fc_w, fc_b,
           _trace=False):
    in_maps = _host_prep(np.asarray(x, dtype=np.float32),
                         np.asarray(proj_w, dtype=np.float32),
                         np.asarray(proj_b, dtype=np.float32),
                         np.asarray(in_proj_w, dtype=np.float32),
                         np.asarray(in_proj_b, dtype=np.float32),
                         np.asarray(out_w, dtype=np.float32),
                         np.asarray(out_b, dtype=np.float32),
                         np.asarray(fc_w, dtype=np.float32),
                         np.asarray(fc_b, dtype=np.float32))
    nc = _get_module()
    res = run_bass_kernel_spmd(nc, in_maps, core_ids=list(range(NCORES)),
                               trace=_trace)
    outs = []
    for core in range(NCORES):
        oc = res.results[core]["out"]                        # [P, NT, E]
        oc = oc.transpose(1, 0, 2).reshape(S_PAD, E)[:S]
        oc = oc.reshape(B_SH, NTOK, E)
        outs.append(oc)
    full = np.concatenate(outs, axis=0)                      # [64, 207, 8]
    if _trace:
        kernel._last_exec_time_ns = res.exec_time_ns
        kernel._last_profile = res.profile_json
    return full.astype(np.float32)
